# revision 11
# baseline (speedup 1.0000x reference)
"""DCGRU cell (DCRNN diffusion-conv GRU) Trainium2 kernel.

Sharding: data-parallel over batch B=64 across 8 NeuronCores (8 batches/core);
supports and weights replicated.

Per-core layout ("orientation A"): activations stored node-major as
(n, f) with n on SBUF partitions (16 tiles of 128) and f = b_local*66 + c in
the free dim (528 cols, padded to 640 where needed).

Diffusion x1 = A @ x runs as PE matmuls with lhsT = A^T tiles (streamed from
DRAM in m-slab layout) and rhs = activation tiles; the Chebyshev recurrence
x2 = 2*A@x1 - x0 keeps this orientation, with the combine fused into one
scalar_tensor_tensor op.

The output projection contracts channels, so features are transposed into
H_T tiles (f on partitions, n free) via big XBAR DMA-transposes from a DRAM
bounce copy of each feature — one (2048,128)->(128,2048) instruction per
(feat, col-tile), since DMA_TRANSPOSE cost is per-instruction ucode on the
issuing sequencer. Per-(b, feat, col-tile) zero-masked weight tiles (built
on host) make every projection matmul a full K=128, base-0 matmul.

All pools are opened once (flat scope) so H_T transposes and projection
work overlap the diffusion of the following step instead of serializing on
recycled SBUF addresses.
"""

import sys

if "/opt/trn_rl_repo" not in sys.path:
    sys.path.insert(0, "/opt/trn_rl_repo")

import numpy as np
import ml_dtypes

import concourse.bass as bass
import concourse.mybir as mybir
import concourse.tile as tile
from concourse import bacc
from concourse.alu_op_type import AluOpType
from concourse.bass_utils import run_bass_kernel_spmd

N_CORES = 8
B = 64
B_LOC = B // N_CORES          # 8
IN_DIM = 2
HID = 64
N = 2048
CIN = IN_DIM + HID            # 66
F = B_LOC * CIN               # 528
FP = 640                      # F padded to multiple of 128
NT = N // 128                 # 16 n-tiles
NFEAT = 5                     # x, A0 x, (2A0^2-1)x, A1 x, (2A1^2-1)x
NCT = FP // 128               # 5 column tiles per feature
CHUNK = 512
NCHUNK = N // CHUNK           # 4

bf16 = mybir.dt.bfloat16
f32 = mybir.dt.float32
AF = mybir.ActivationFunctionType


def _cts(b):
    """H_T column-tiles overlapped by batch b's 66 f-rows."""
    lo, hi = CIN * b, CIN * b + CIN - 1
    return list(range(lo // 128, hi // 128 + 1))


# piece list shared by host weight construction and kernel builder
PIECES = [(b, ft, ct) for b in range(B_LOC) for ft in range(NFEAT) for ct in _cts(b)]
PIDX = {p: i for i, p in enumerate(PIECES)}
NPIECE = len(PIECES)  # 60


def _build_nc():
    nc = bacc.Bacc("TRN2", target_bir_lowering=False, debug=False,
                   num_devices=N_CORES)

    x0_d = nc.dram_tensor("x0", [NT, 128, F], bf16, kind="ExternalInput")
    x0t_d = nc.dram_tensor("x0t", [NCT, 128, N], bf16, kind="ExternalInput")
    a_d = [nc.dram_tensor("a0s", [NT, 128, N], bf16, kind="ExternalInput"),
           nc.dram_tensor("a1s", [NT, 128, N], bf16, kind="ExternalInput")]
    hxp_d = nc.dram_tensor("hxp", [B_LOC // 2, 128, N], f32, kind="ExternalInput")
    wru_d = nc.dram_tensor("wru", [128, NPIECE * 2 * HID], bf16, kind="ExternalInput")
    wc_d = nc.dram_tensor("wc", [128, NPIECE * HID], bf16, kind="ExternalInput")
    bru_d = nc.dram_tensor("bru", [2 * HID], f32, kind="ExternalInput")
    bc_d = nc.dram_tensor("bc", [HID], f32, kind="ExternalInput")
    id_d = nc.dram_tensor("ident", [64, 64], bf16, kind="ExternalInput")
    out_d = nc.dram_tensor("out", [B_LOC, HID, N], f32, kind="ExternalOutput")

    # DRAM bounce copies of features, n-major (2048, 640), for XBAR transposes
    fdr = {ft: nc.dram_tensor(f"fdr{ft}", [N, FP], bf16) for ft in range(NFEAT)}
    # u gate spilled to DRAM between conv1 and conv2
    u_dr = nc.dram_tensor("u_dr", [B_LOC, HID, N], bf16)

    with tile.TileContext(nc) as tc:
        with (
            tc.tile_pool(name="persist", bufs=1) as persist,
            tc.tile_pool(name="f13", bufs=1) as f13p,
            tc.tile_pool(name="aslab", bufs=2) as ap_pool,
            tc.tile_pool(name="fst", bufs=3) as fstp,
            tc.tile_pool(name="ht", bufs=1) as htp,
            tc.tile_pool(name="stage", bufs=2) as stg,
            tc.tile_pool(name="dpsum", bufs=2, space="PSUM") as dps,
            tc.tile_pool(name="ppsum", bufs=2, space="PSUM") as pps,
            tc.tile_pool(name="tpsum", bufs=2, space="PSUM") as tps,
        ):
            # F0: persistent recurrence base, full 640 wide (zero tail)
            Ft = {}
            for m in range(NT):
                Ft[(0, m)] = persist.tile([128, FP], bf16,
                                          name=f"F0m{m}", tag=f"F0m{m}")
                nc.gpsimd.memset(Ft[(0, m)][:, F:FP], 0.0)
                nc.sync.dma_start(Ft[(0, m)][:, 0:F], x0_d[m])
            # F1/F3: 528 wide, rewritten each conv
            for ft in (1, 3):
                for m in range(NT):
                    Ft[(ft, m)] = f13p.tile([128, F], bf16,
                                            name=f"F{ft}m{m}", tag=f"F{ft}m{m}")

            # one-time zero of the fdr col tails (528:640) for ft 1..4
            ztail = persist.tile([128, FP - F], bf16)
            nc.gpsimd.memset(ztail[:], 0.0)
            for ft in range(1, NFEAT):
                for m in range(NT):
                    nc.sync.dma_start(fdr[ft][m * 128:(m + 1) * 128, F:FP],
                                      ztail[:])

            bru_lo = persist.tile([HID, 1], f32)
            bru_hi = persist.tile([HID, 1], f32)
            bc_t = persist.tile([HID, 1], f32)
            ident = persist.tile([64, 64], bf16)
            nc.sync.dma_start(bru_lo[:], bru_d[0:HID, None])
            nc.sync.dma_start(bru_hi[:], bru_d[HID:2 * HID, None])
            nc.sync.dma_start(bc_t[:], bc_d[:, None])
            nc.sync.dma_start(ident[:], id_d[:])

            # packed projection weights, both convs resident
            w_ru = persist.tile([128, NPIECE * 2 * HID], bf16)
            w_c = persist.tile([128, NPIECE * HID], bf16)
            nc.sync.dma_start(w_ru[:], wru_d[:])
            nc.sync.dma_start(w_c[:], wc_d[:])

            for conv in range(2):
                # ---------------- diffusion ----------------
                # (A-dram-idx, src feat, dst feat, fused-combine?)
                apps = [(0, 0, 1, False), (1, 0, 3, False),
                        (0, 1, 2, True), (1, 3, 4, True)]
                for ai, src, dst, fused in apps:
                    for m in range(NT):
                        aslab = ap_pool.tile([128, N], bf16, tag="aslab")
                        nc.sync.dma_start(aslab[:], a_d[ai][m])
                        p0 = dps.tile([128, 264], f32, tag="dp0")
                        p1 = dps.tile([128, 264], f32, tag="dp1")
                        src_t = Ft[(src, 0)]  # placeholder for type
                        for k in range(NT):
                            st, sp = k == 0, k == NT - 1
                            lhs = aslab[:, k * 128:(k + 1) * 128]
                            nc.tensor.matmul(p0[:], lhs,
                                             Ft[(src, k)][:, 0:264],
                                             start=st, stop=sp)
                            nc.tensor.matmul(p1[:], lhs,
                                             Ft[(src, k)][:, 264:528],
                                             start=st, stop=sp)
                        if fused:
                            # F_dst = 2 * (A @ F_src) - F0, to DRAM only
                            fs = fstp.tile([128, F], bf16, tag="fstage")
                            nc.vector.scalar_tensor_tensor(
                                fs[:, 0:264], p0[:], 2.0,
                                Ft[(0, m)][:, 0:264],
                                AluOpType.mult, AluOpType.subtract)
                            nc.vector.scalar_tensor_tensor(
                                fs[:, 264:528], p1[:], 2.0,
                                Ft[(0, m)][:, 264:528],
                                AluOpType.mult, AluOpType.subtract)
                            nc.sync.dma_start(
                                fdr[dst][m * 128:(m + 1) * 128, 0:F], fs[:])
                        else:
                            ftile = Ft[(dst, m)]
                            nc.scalar.copy(ftile[:, 0:264], p0[:])
                            nc.scalar.copy(ftile[:, 264:528], p1[:])
                            nc.sync.dma_start(
                                fdr[dst][m * 128:(m + 1) * 128, 0:F], ftile[:])

                # ---------------- projection ----------------
                O = 2 * HID if conv == 0 else HID
                w_t = w_ru if conv == 0 else w_c

                ht = {}
                for ft in range(NFEAT):
                    for ct in range(NCT):
                        h = htp.tile([128, N], bf16, tag=f"ht{ft}_{ct}",
                                     name=f"ht{ft}_{ct}_{conv}")
                        if conv == 0 and ft == 0:
                            nc.sync.dma_start(h[:], x0t_d[ct])
                        else:
                            nc.scalar.dma_start(
                                h[:], fdr[ft][:, ct * 128:(ct + 1) * 128],
                                transpose=True)
                        ht[(ft, ct)] = h

                c_st = None
                for ch in range(NCHUNK):
                    nsl = slice(ch * CHUNK, (ch + 1) * CHUNK)
                    for b in range(B_LOC):
                        pp = pps.tile([O, CHUNK], f32, tag="pp")
                        seq = [(ft, ct) for ft in range(NFEAT)
                               for ct in _cts(b)]
                        for i, (ft, ct) in enumerate(seq):
                            pi = PIDX[(b, ft, ct)]
                            nc.tensor.matmul(
                                pp[:], w_t[:, pi * O:(pi + 1) * O],
                                ht[(ft, ct)][:, nsl],
                                start=(i == 0), stop=(i == len(seq) - 1))
                        if conv == 0:
                            # value = sigmoid(. + b_ru); rows 0:64=r, 64:128=u
                            r_st = stg.tile([HID, CHUNK], bf16, tag="rst")
                            nc.scalar.activation(r_st[:], pp[0:HID, :],
                                                 AF.Sigmoid, bias=bru_lo[:])
                            u_st = stg.tile([HID, CHUNK], bf16, tag="ust")
                            nc.scalar.activation(u_st[:], pp[HID:2 * HID, :],
                                                 AF.Sigmoid, bias=bru_hi[:])
                            nc.sync.dma_start(u_dr[b][:, nsl], u_st[:])
                            # X0 hx-cols *= r^T (PE transpose of r blocks)
                            for j in range(4):
                                rT = tps.tile([128, HID], bf16, tag="rT")
                                nc.tensor.transpose(
                                    rT[:], r_st[:, j * 128:(j + 1) * 128],
                                    ident[:])
                                m = 4 * ch + j
                                cols = slice(CIN * b + IN_DIM, CIN * (b + 1))
                                f0 = Ft[(0, m)]
                                nc.vector.tensor_mul(f0[:, cols],
                                                     f0[:, cols], rT[:])
                        else:
                            # C = tanh(. + b_c); gates per b-pair
                            half = b % 2
                            if half == 0:
                                c_st = stg.tile([128, CHUNK], bf16, tag="cst")
                            nc.scalar.activation(
                                c_st[half * HID:(half + 1) * HID, :],
                                pp[0:HID, :], AF.Tanh, bias=bc_t[:])
                            if half == 1:
                                # out = C + u * (hx - C), reusing staging
                                t4 = b // 2
                                hx_t = stg.tile([128, CHUNK], f32, tag="hxs")
                                nc.sync.dma_start(hx_t[:], hxp_d[t4][:, nsl])
                                u_t = stg.tile([128, CHUNK], bf16, tag="ut")
                                nc.sync.dma_start(
                                    u_t[:],
                                    u_dr.rearrange("b h n -> (b h) n")[
                                        t4 * 128:(t4 + 1) * 128, nsl])
                                t1 = stg.tile([128, CHUNK], f32, tag="t1")
                                nc.vector.tensor_sub(t1[:], hx_t[:], c_st[:])
                                nc.vector.tensor_mul(hx_t[:], u_t[:], t1[:])
                                nc.vector.tensor_add(t1[:], c_st[:], hx_t[:])
                                nc.sync.dma_start(
                                    out_d.rearrange("b h n -> (b h) n")[
                                        t4 * 128:(t4 + 1) * 128, nsl],
                                    t1[:])
                    if conv == 0:
                        # X0' complete for this chunk's m-tiles -> DRAM
                        # bounce for conv2's F0 transposes
                        for j in range(4):
                            m = 4 * ch + j
                            nc.sync.dma_start(
                                fdr[0][m * 128:(m + 1) * 128, :],
                                Ft[(0, m)][:])

    nc.compile()
    return nc


_NC_CACHE = None


def _get_nc():
    global _NC_CACHE
    if _NC_CACHE is None:
        _NC_CACHE = _build_nc()
    return _NC_CACHE


def _slabify(a):
    """(N, N) support -> (NT, 128, N) m-slab layout of A^T for lhsT tiles."""
    return np.ascontiguousarray(
        a.reshape(NT, 128, NT, 128).transpose(0, 3, 2, 1).reshape(NT, 128, N)
    ).astype(ml_dtypes.bfloat16)


def _expand_w(w):
    """(330, O) -> (128, NPIECE*O) zero-masked per-piece lhsT tiles,
    partition-major for a direct DMA into the packed weight tile."""
    ow = w.shape[1]
    out = np.zeros((NPIECE, 128, ow), np.float32)
    for i, (b, ft, ct) in enumerate(PIECES):
        for p in range(128):
            g = 128 * ct + p
            if CIN * b <= g < CIN * (b + 1) and g < F:
                out[i, p] = w[CIN * ft + (g - CIN * b)]
    return np.ascontiguousarray(
        out.transpose(1, 0, 2).reshape(128, NPIECE * ow)
    ).astype(ml_dtypes.bfloat16)


def _make_in_maps(inputs, hx, support0, support1, W_ru, b_ru, W_c, b_c):
    a0s = _slabify(np.asarray(support0, np.float32))
    a1s = _slabify(np.asarray(support1, np.float32))
    wru = _expand_w(np.asarray(W_ru, np.float32))
    wc = _expand_w(np.asarray(W_c, np.float32))
    bru = np.asarray(b_ru, np.float32)
    bc = np.asarray(b_c, np.float32)
    ident = np.eye(64, dtype=np.float32).astype(ml_dtypes.bfloat16)

    cc = np.concatenate([np.asarray(inputs, np.float32),
                         np.asarray(hx, np.float32)], axis=1)  # (B, 66, N)

    in_maps = []
    for ci in range(N_CORES):
        sl = slice(ci * B_LOC, (ci + 1) * B_LOC)
        ccl = cc[sl]                                   # (8, 66, N)
        x0 = np.ascontiguousarray(
            ccl.transpose(2, 0, 1).reshape(NT, 128, F)
        ).astype(ml_dtypes.bfloat16)
        x0t = np.zeros((NCT * 128, N), np.float32)
        x0t[0:F] = ccl.reshape(F, N)
        x0t = x0t.reshape(NCT, 128, N).astype(ml_dtypes.bfloat16)
        hxp = np.ascontiguousarray(
            np.asarray(hx[sl], np.float32).reshape(B_LOC // 2, 128, N))
        in_maps.append({"x0": x0, "x0t": x0t, "a0s": a0s, "a1s": a1s,
                        "hxp": hxp, "wru": wru, "wc": wc, "bru": bru,
                        "bc": bc, "ident": ident})
    return in_maps


def kernel(inputs, hx, support0, support1, W_ru, b_ru, W_c, b_c):
    nc = _get_nc()
    in_maps = _make_in_maps(inputs, hx, support0, support1,
                            W_ru, b_ru, W_c, b_c)
    res = run_bass_kernel_spmd(nc, in_maps, list(range(N_CORES)), trace=False)

    out = np.empty((B, HID, N), np.float32)
    for ci in range(N_CORES):
        out[ci * B_LOC:(ci + 1) * B_LOC] = res.results[ci]["out"]
    return out


# revision 12
# speedup vs baseline: 1.1760x; 1.1760x over previous
"""DCGRU cell (DCRNN diffusion-conv GRU) Trainium2 kernel.

Sharding: data-parallel over batch B=64 across 8 NeuronCores (8 batches/core);
supports and weights replicated.

Per-core layout ("orientation A"): activations stored node-major as
(n, f) with n on SBUF partitions (16 tiles of 128) and f = b_local*66 + c in
the free dim (528 cols, padded to 640 where needed).

Diffusion x1 = A @ x runs as PE matmuls with lhsT = A^T tiles (streamed from
DRAM in m-slab layout) and rhs = activation tiles; the Chebyshev recurrence
x2 = 2*A@x1 - x0 keeps this orientation, with the combine fused into one
scalar_tensor_tensor op.

The output projection contracts channels, so features are transposed into
H_T tiles (f on partitions, n free) via big XBAR DMA-transposes from a DRAM
bounce copy of each feature — one (2048,128)->(128,2048) instruction per
(feat, col-tile), since DMA_TRANSPOSE cost is per-instruction ucode on the
issuing sequencer. Per-(b, feat, col-tile) zero-masked weight tiles (built
on host) make every projection matmul a full K=128, base-0 matmul.

All pools are opened once (flat scope) so H_T transposes and projection
work overlap the diffusion of the following step instead of serializing on
recycled SBUF addresses.
"""

import sys

if "/opt/trn_rl_repo" not in sys.path:
    sys.path.insert(0, "/opt/trn_rl_repo")

import numpy as np
import ml_dtypes

import concourse.bass as bass
import concourse.mybir as mybir
import concourse.tile as tile
from concourse import bacc
from concourse.alu_op_type import AluOpType
from concourse.bass_utils import run_bass_kernel_spmd

N_CORES = 8
B = 64
B_LOC = B // N_CORES          # 8
IN_DIM = 2
HID = 64
N = 2048
CIN = IN_DIM + HID            # 66
F = B_LOC * CIN               # 528
FP = 640                      # F padded to multiple of 128
NT = N // 128                 # 16 n-tiles
NFEAT = 5                     # x, A0 x, (2A0^2-1)x, A1 x, (2A1^2-1)x
NCT = FP // 128               # 5 column tiles per feature
CHUNK = 512
NCHUNK = N // CHUNK           # 4

bf16 = mybir.dt.bfloat16
f32 = mybir.dt.float32
AF = mybir.ActivationFunctionType


def _cts(b):
    """H_T column-tiles overlapped by batch b's 66 f-rows."""
    lo, hi = CIN * b, CIN * b + CIN - 1
    return list(range(lo // 128, hi // 128 + 1))


# piece list shared by host weight construction and kernel builder
PIECES = [(b, ft, ct) for b in range(B_LOC) for ft in range(NFEAT) for ct in _cts(b)]
PIDX = {p: i for i, p in enumerate(PIECES)}
NPIECE = len(PIECES)  # 60


def _build_nc():
    nc = bacc.Bacc("TRN2", target_bir_lowering=False, debug=False,
                   num_devices=N_CORES)

    x0_d = nc.dram_tensor("x0", [NT, 128, F], bf16, kind="ExternalInput")
    x0t_d = nc.dram_tensor("x0t", [NCT, 128, N], bf16, kind="ExternalInput")
    a_d = [nc.dram_tensor("a0s", [NT, 128, N], bf16, kind="ExternalInput"),
           nc.dram_tensor("a1s", [NT, 128, N], bf16, kind="ExternalInput")]
    hxp_d = nc.dram_tensor("hxp", [B_LOC // 2, 128, N], f32, kind="ExternalInput")
    wru_d = nc.dram_tensor("wru", [128, NPIECE * 2 * HID], bf16, kind="ExternalInput")
    wc_d = nc.dram_tensor("wc", [128, NPIECE * HID], bf16, kind="ExternalInput")
    bru_d = nc.dram_tensor("bru", [2 * HID], f32, kind="ExternalInput")
    bc_d = nc.dram_tensor("bc", [HID], f32, kind="ExternalInput")
    id_d = nc.dram_tensor("ident", [64, 64], bf16, kind="ExternalInput")
    out_d = nc.dram_tensor("out", [B_LOC, HID, N], f32, kind="ExternalOutput")

    # DRAM bounce copies of features, n-major (2048, 640), for XBAR transposes
    fdr = {ft: nc.dram_tensor(f"fdr{ft}", [N, FP], bf16) for ft in range(NFEAT)}
    # u gate spilled to DRAM between conv1 and conv2
    u_dr = nc.dram_tensor("u_dr", [B_LOC, HID, N], bf16)

    with tile.TileContext(nc) as tc:
        with (
            tc.tile_pool(name="persist", bufs=1) as persist,
            tc.tile_pool(name="f13", bufs=1) as f13p,
            tc.tile_pool(name="aslab", bufs=3) as ap_pool,
            tc.tile_pool(name="fst", bufs=3) as fstp,
            tc.tile_pool(name="ht", bufs=1) as htp,
            tc.tile_pool(name="stage", bufs=2) as stg,
            tc.tile_pool(name="dpsum", bufs=2, space="PSUM") as dps,
            tc.tile_pool(name="ppsum", bufs=2, space="PSUM") as pps,
            tc.tile_pool(name="tpsum", bufs=2, space="PSUM") as tps,
        ):
            # F0: persistent recurrence base, full 640 wide (zero tail)
            Ft = {}
            for m in range(NT):
                Ft[(0, m)] = persist.tile([128, FP], bf16,
                                          name=f"F0m{m}", tag=f"F0m{m}")
                nc.gpsimd.memset(Ft[(0, m)][:, F:FP], 0.0)
                nc.sync.dma_start(Ft[(0, m)][:, 0:F], x0_d[m])
            # F1/F3: 528 wide, rewritten each conv
            for ft in (1, 3):
                for m in range(NT):
                    Ft[(ft, m)] = f13p.tile([128, F], bf16,
                                            name=f"F{ft}m{m}", tag=f"F{ft}m{m}")

            # one-time zero of the fdr col tails (528:640) for ft 1..4
            ztail = persist.tile([128, FP - F], bf16)
            nc.gpsimd.memset(ztail[:], 0.0)
            for ft in range(1, NFEAT):
                for m in range(NT):
                    nc.scalar.dma_start(fdr[ft][m * 128:(m + 1) * 128, F:FP],
                                        ztail[:])

            bru_lo = persist.tile([HID, 1], f32)
            bru_hi = persist.tile([HID, 1], f32)
            bc_t = persist.tile([HID, 1], f32)
            ident = persist.tile([64, 64], bf16)
            nc.sync.dma_start(bru_lo[:], bru_d[0:HID, None])
            nc.sync.dma_start(bru_hi[:], bru_d[HID:2 * HID, None])
            nc.sync.dma_start(bc_t[:], bc_d[:, None])
            nc.sync.dma_start(ident[:], id_d[:])

            # packed projection weights, both convs resident
            w_ru = persist.tile([128, NPIECE * 2 * HID], bf16)
            w_c = persist.tile([128, NPIECE * HID], bf16)
            nc.sync.dma_start(w_ru[:], wru_d[:])
            nc.sync.dma_start(w_c[:], wc_d[:])

            for conv in range(2):
                # ---------------- diffusion ----------------
                # (A-dram-idx, src feat, dst feat, fused-combine?)
                apps = [(0, 0, 1, False), (1, 0, 3, False),
                        (0, 1, 2, True), (1, 3, 4, True)]
                for ai, src, dst, fused in apps:
                    for m in range(NT):
                        aslab = ap_pool.tile([128, N], bf16, tag="aslab")
                        nc.sync.dma_start(aslab[:], a_d[ai][m])
                        p0 = dps.tile([128, 264], f32, tag="dp0")
                        p1 = dps.tile([128, 264], f32, tag="dp1")
                        src_t = Ft[(src, 0)]  # placeholder for type
                        for k in range(NT):
                            st, sp = k == 0, k == NT - 1
                            lhs = aslab[:, k * 128:(k + 1) * 128]
                            nc.tensor.matmul(p0[:], lhs,
                                             Ft[(src, k)][:, 0:264],
                                             start=st, stop=sp)
                            nc.tensor.matmul(p1[:], lhs,
                                             Ft[(src, k)][:, 264:528],
                                             start=st, stop=sp)
                        if fused:
                            # F_dst = 2 * (A @ F_src) - F0, to DRAM only
                            fs = fstp.tile([128, F], bf16, tag="fstage")
                            nc.vector.scalar_tensor_tensor(
                                fs[:, 0:264], p0[:], 2.0,
                                Ft[(0, m)][:, 0:264],
                                AluOpType.mult, AluOpType.subtract)
                            nc.vector.scalar_tensor_tensor(
                                fs[:, 264:528], p1[:], 2.0,
                                Ft[(0, m)][:, 264:528],
                                AluOpType.mult, AluOpType.subtract)
                            nc.scalar.dma_start(
                                fdr[dst][m * 128:(m + 1) * 128, 0:F], fs[:])
                        else:
                            ftile = Ft[(dst, m)]
                            nc.scalar.copy(ftile[:, 0:264], p0[:])
                            nc.scalar.copy(ftile[:, 264:528], p1[:])
                            nc.scalar.dma_start(
                                fdr[dst][m * 128:(m + 1) * 128, 0:F], ftile[:])

                # ---------------- projection ----------------
                O = 2 * HID if conv == 0 else HID
                w_t = w_ru if conv == 0 else w_c

                ht = {}
                for ft in range(NFEAT):
                    for ct in range(NCT):
                        h = htp.tile([128, N], bf16, tag=f"ht{ft}_{ct}",
                                     name=f"ht{ft}_{ct}_{conv}")
                        if conv == 0 and ft == 0:
                            nc.sync.dma_start(h[:], x0t_d[ct])
                        else:
                            nc.scalar.dma_start(
                                h[:], fdr[ft][:, ct * 128:(ct + 1) * 128],
                                transpose=True)
                        ht[(ft, ct)] = h

                c_st = None
                for ch in range(NCHUNK):
                    nsl = slice(ch * CHUNK, (ch + 1) * CHUNK)
                    for b in range(B_LOC):
                        pp = pps.tile([O, CHUNK], f32, tag="pp")
                        seq = [(ft, ct) for ft in range(NFEAT)
                               for ct in _cts(b)]
                        for i, (ft, ct) in enumerate(seq):
                            pi = PIDX[(b, ft, ct)]
                            nc.tensor.matmul(
                                pp[:], w_t[:, pi * O:(pi + 1) * O],
                                ht[(ft, ct)][:, nsl],
                                start=(i == 0), stop=(i == len(seq) - 1))
                        if conv == 0:
                            # value = sigmoid(. + b_ru); rows 0:64=r, 64:128=u
                            r_st = stg.tile([HID, CHUNK], bf16, tag="rst")
                            nc.scalar.activation(r_st[:], pp[0:HID, :],
                                                 AF.Sigmoid, bias=bru_lo[:])
                            u_st = stg.tile([HID, CHUNK], bf16, tag="ust")
                            nc.scalar.activation(u_st[:], pp[HID:2 * HID, :],
                                                 AF.Sigmoid, bias=bru_hi[:])
                            nc.sync.dma_start(u_dr[b][:, nsl], u_st[:])
                            # X0 hx-cols *= r^T (PE transpose of r blocks)
                            for j in range(4):
                                rT = tps.tile([128, HID], bf16, tag="rT")
                                nc.tensor.transpose(
                                    rT[:], r_st[:, j * 128:(j + 1) * 128],
                                    ident[:])
                                m = 4 * ch + j
                                cols = slice(CIN * b + IN_DIM, CIN * (b + 1))
                                f0 = Ft[(0, m)]
                                nc.vector.tensor_mul(f0[:, cols],
                                                     f0[:, cols], rT[:])
                        else:
                            # C = tanh(. + b_c); gates per b-pair
                            half = b % 2
                            if half == 0:
                                c_st = stg.tile([128, CHUNK], bf16, tag="cst")
                            nc.scalar.activation(
                                c_st[half * HID:(half + 1) * HID, :],
                                pp[0:HID, :], AF.Tanh, bias=bc_t[:])
                            if half == 1:
                                # out = C + u * (hx - C), reusing staging
                                t4 = b // 2
                                hx_t = stg.tile([128, CHUNK], f32, tag="hxs")
                                nc.sync.dma_start(hx_t[:], hxp_d[t4][:, nsl])
                                u_t = stg.tile([128, CHUNK], bf16, tag="ut")
                                nc.sync.dma_start(
                                    u_t[:],
                                    u_dr.rearrange("b h n -> (b h) n")[
                                        t4 * 128:(t4 + 1) * 128, nsl])
                                t1 = stg.tile([128, CHUNK], f32, tag="t1")
                                nc.vector.tensor_sub(t1[:], hx_t[:], c_st[:])
                                nc.vector.tensor_mul(hx_t[:], u_t[:], t1[:])
                                nc.vector.tensor_add(t1[:], c_st[:], hx_t[:])
                                nc.sync.dma_start(
                                    out_d.rearrange("b h n -> (b h) n")[
                                        t4 * 128:(t4 + 1) * 128, nsl],
                                    t1[:])
                    if conv == 0:
                        # X0' complete for this chunk's m-tiles -> DRAM
                        # bounce for conv2's F0 transposes
                        for j in range(4):
                            m = 4 * ch + j
                            nc.scalar.dma_start(
                                fdr[0][m * 128:(m + 1) * 128, :],
                                Ft[(0, m)][:])

    nc.compile()
    return nc


_NC_CACHE = None


def _get_nc():
    global _NC_CACHE
    if _NC_CACHE is None:
        _NC_CACHE = _build_nc()
    return _NC_CACHE


def _slabify(a):
    """(N, N) support -> (NT, 128, N) m-slab layout of A^T for lhsT tiles."""
    return np.ascontiguousarray(
        a.reshape(NT, 128, NT, 128).transpose(0, 3, 2, 1).reshape(NT, 128, N)
    ).astype(ml_dtypes.bfloat16)


def _expand_w(w):
    """(330, O) -> (128, NPIECE*O) zero-masked per-piece lhsT tiles,
    partition-major for a direct DMA into the packed weight tile."""
    ow = w.shape[1]
    out = np.zeros((NPIECE, 128, ow), np.float32)
    for i, (b, ft, ct) in enumerate(PIECES):
        for p in range(128):
            g = 128 * ct + p
            if CIN * b <= g < CIN * (b + 1) and g < F:
                out[i, p] = w[CIN * ft + (g - CIN * b)]
    return np.ascontiguousarray(
        out.transpose(1, 0, 2).reshape(128, NPIECE * ow)
    ).astype(ml_dtypes.bfloat16)


def _make_in_maps(inputs, hx, support0, support1, W_ru, b_ru, W_c, b_c):
    a0s = _slabify(np.asarray(support0, np.float32))
    a1s = _slabify(np.asarray(support1, np.float32))
    wru = _expand_w(np.asarray(W_ru, np.float32))
    wc = _expand_w(np.asarray(W_c, np.float32))
    bru = np.asarray(b_ru, np.float32)
    bc = np.asarray(b_c, np.float32)
    ident = np.eye(64, dtype=np.float32).astype(ml_dtypes.bfloat16)

    cc = np.concatenate([np.asarray(inputs, np.float32),
                         np.asarray(hx, np.float32)], axis=1)  # (B, 66, N)

    in_maps = []
    for ci in range(N_CORES):
        sl = slice(ci * B_LOC, (ci + 1) * B_LOC)
        ccl = cc[sl]                                   # (8, 66, N)
        x0 = np.ascontiguousarray(
            ccl.transpose(2, 0, 1).reshape(NT, 128, F)
        ).astype(ml_dtypes.bfloat16)
        x0t = np.zeros((NCT * 128, N), np.float32)
        x0t[0:F] = ccl.reshape(F, N)
        x0t = x0t.reshape(NCT, 128, N).astype(ml_dtypes.bfloat16)
        hxp = np.ascontiguousarray(
            np.asarray(hx[sl], np.float32).reshape(B_LOC // 2, 128, N))
        in_maps.append({"x0": x0, "x0t": x0t, "a0s": a0s, "a1s": a1s,
                        "hxp": hxp, "wru": wru, "wc": wc, "bru": bru,
                        "bc": bc, "ident": ident})
    return in_maps


def kernel(inputs, hx, support0, support1, W_ru, b_ru, W_c, b_c):
    nc = _get_nc()
    in_maps = _make_in_maps(inputs, hx, support0, support1,
                            W_ru, b_ru, W_c, b_c)
    res = run_bass_kernel_spmd(nc, in_maps, list(range(N_CORES)), trace=False)

    out = np.empty((B, HID, N), np.float32)
    for ci in range(N_CORES):
        out[ci * B_LOC:(ci + 1) * B_LOC] = res.results[ci]["out"]
    return out
